# revision 1
# baseline (speedup 1.0000x reference)
"""Trainium2 Bass kernel for nn_AutoEncoder_51642686767592.

Data-parallel over the batch dim across 8 NeuronCores. Global reductions
(median of row sums, global norm stats, BatchNorm batch stats) via on-device
collectives (AllGather + 3 small AllReduces).

Math notes (vs reference):
  preprocess: s = x.sum(1); med = lower-median(s); norm = log(x/(s/med) + 1)
  h = (norm - mean)/std(ddof=1)       <- folded into BN1:
  BN1(h@W_in + b_in) == (A - muA) * rsqrt(varA + sigma^2*eps) * g1 + bt1
      where A = norm@W_in (no bias), sigma^2 = global var(norm, ddof=1).
  b_in/b_enc/b_dec and the global mean all cancel inside BatchNorm.
  Head biases are applied via a ones-row (K=65) in the head matmuls.
"""
import numpy as np

import concourse.bacc as bacc
import concourse.mybir as mybir
import concourse.tile as tile
from concourse.bass_utils import run_bass_kernel_spmd

F32 = mybir.dt.float32
ALU = mybir.AluOpType
ACTF = mybir.ActivationFunctionType
AX = mybir.AxisListType

N_CORES = 8
B, D = 16384, 4096
H1, H2 = 64, 32
R = B // N_CORES          # rows per core = 2048
NT = R // 128             # row tiles per core = 16
NC_ = D // 128            # d chunks = 32
NB = R // 512             # 512-row blocks = 4
N_ELEMS = float(B * D)    # norm element count
MED_RANK = 8192.0         # count(s <= t) >= 8192  <=>  t >= lower median
BIS_ITERS = 7

_CACHE = {}


def _build():
    nc = bacc.Bacc("TRN2", target_bir_lowering=False, debug=False,
                   num_devices=N_CORES)

    # ---- kernel I/O ----
    x_d = nc.dram_tensor("x", [R, D], F32, kind="ExternalInput")
    win_d = nc.dram_tensor("W_in", [D, H1], F32, kind="ExternalInput")
    wenc_d = nc.dram_tensor("W_enc", [H1, H2], F32, kind="ExternalInput")
    wdec_d = nc.dram_tensor("W_dec", [H2, H1], F32, kind="ExternalInput")
    wh_d = [nc.dram_tensor(n, [H1, D], F32, kind="ExternalInput")
            for n in ("W_pi", "W_m", "W_th")]
    bh_d = [nc.dram_tensor(n, [D], F32, kind="ExternalInput")
            for n in ("b_pi", "b_m", "b_th")]
    g_d = [nc.dram_tensor(n, [sz], F32, kind="ExternalInput")
           for n, sz in (("g1", H1), ("bt1", H1), ("g2", H2), ("bt2", H2),
                         ("g3", H1), ("bt3", H1))]
    ident_d = nc.dram_tensor("ident", [128, 128], F32, kind="ExternalInput")
    ones_d = nc.dram_tensor("ones", [128, 128], F32, kind="ExternalInput")
    j15_d = nc.dram_tensor("j15", [128, 15], F32, kind="ExternalInput")

    out_d = [nc.dram_tensor(n, [R, D], F32, kind="ExternalOutput")
             for n in ("PI", "M", "TH")]

    with tile.TileContext(nc) as tc:
        with tc.tile_pool(name="wpool", bufs=1) as wp, \
             tc.tile_pool(name="spool", bufs=1) as sp, \
             tc.tile_pool(name="dram", bufs=1, space="DRAM") as dp:

            # ---- constants / weights resident in SBUF ----
            ident = wp.tile([128, 128], F32)
            nc.sync.dma_start(out=ident[:], in_=ident_d[:])
            ones = wp.tile([128, 128], F32)
            nc.sync.dma_start(out=ones[:], in_=ones_d[:])
            j15 = wp.tile([128, 15], F32)
            nc.sync.dma_start(out=j15[:], in_=j15_d[:])
            wi = wp.tile([128, NC_, H1], F32)   # W_in as [p, chunk, 64]
            nc.sync.dma_start(out=wi[:],
                              in_=win_d[:].rearrange("(c p) k -> p c k", p=128))
            wenc = wp.tile([H1, H2], F32)
            nc.sync.dma_start(out=wenc[:], in_=wenc_d[:])
            wdec = wp.tile([H2, H1], F32)
            nc.sync.dma_start(out=wdec[:], in_=wdec_d[:])
            whe = wp.tile([H1 + 1, 3, D], F32)  # [W_head; b_head] stacked
            for h in range(3):
                nc.sync.dma_start(out=whe[0:H1, h, :], in_=wh_d[h][:])
                nc.sync.dma_start(out=whe[H1:H1 + 1, h, :],
                                  in_=bh_d[h][:].rearrange("(p f) -> p f", p=1))
            gbt = []
            for t_d in g_d:
                sz = t_d.shape[0]
                tt = wp.tile([sz, 1], F32, name=f"c_{t_d.name}")
                nc.sync.dma_start(out=tt[:],
                                  in_=t_d[:].rearrange("(p f) -> p f", f=1))
                gbt.append(tt)
            g1t, bt1t, g2t, bt2t, g3t, bt3t = gbt

            # ---- stat tiles ----
            svals = sp.tile([128, NT], F32)      # per-row sums (row = p + 128*t)
            rcp_s = sp.tile([128, NT], F32)
            rcp_sp = sp.tile([128, NT], F32)     # med / s
            s_all = sp.tile([128, 128], F32)     # all 16384 row sums (any order)
            nsums = sp.tile([128, NT * 8], F32)  # per-(tile,group) norm sums
            nsq = sp.tile([128, NT * 8], F32)    # per-(tile,group) norm^2 sums

            # ============ PASS 1: row sums ============
            with tc.tile_pool(name="xpool", bufs=3) as xp:
                for t in range(NT):
                    xt = xp.tile([128, D], F32, tag="x")
                    nc.sync.dma_start(out=xt[:], in_=x_d[t * 128:(t + 1) * 128, :])
                    nc.vector.tensor_reduce(svals[:, t:t + 1], xt[:],
                                            axis=AX.X, op=ALU.add)
                nc.vector.reciprocal(rcp_s[:], svals[:])

                # ---- AllGather row sums ----
                sb_in = dp.tile([R], F32)
                sb_out = dp.tile([B], F32, addr_space="Shared")
                nc.sync.dma_start(out=sb_in[:].rearrange("(p t) -> p t", p=128),
                                  in_=svals[:])
                nc.gpsimd.collective_compute(
                    "AllGather", ALU.bypass,
                    replica_groups=[list(range(N_CORES))],
                    ins=[sb_in.opt()], outs=[sb_out.opt()])
                nc.sync.dma_start(out=s_all[:],
                                  in_=sb_out[:].rearrange("(p f) -> p f", p=128))

                # ============ median: 7 rounds of 16-ary search ============
                with tc.tile_pool(name="bis", bufs=1) as bp, \
                     tc.tile_pool(name="bps", bufs=1, space="PSUM") as bps:
                    lo = bp.tile([128, 1], F32)
                    w16 = bp.tile([128, 1], F32)
                    nc.vector.memset(lo[:], 0.0)
                    nc.vector.memset(w16[:], float(D) / 16.0)
                    thr = bp.tile([128, 15], F32)
                    cnt = bp.tile([128, 15], F32)
                    cscr = bp.tile([128, 2, 128], F32)
                    pred = bp.tile([128, 15], F32)
                    idx = bp.tile([128, 1], F32)
                    step = bp.tile([128, 1], F32)
                    med = bp.tile([128, 1], F32)
                    for it in range(BIS_ITERS):
                        nc.vector.tensor_scalar(thr[:], j15[:], w16[:], lo[:],
                                                op0=ALU.mult, op1=ALU.add)
                        for j in range(15):
                            nc.vector.tensor_scalar(
                                cscr[:, j % 2, :], s_all[:], thr[:, j:j + 1],
                                None, op0=ALU.is_le, op1=ALU.add,
                                accum_out=cnt[:, j:j + 1])
                        pcnt = bps.tile([128, 15], F32, tag="pcnt")
                        nc.tensor.matmul(pcnt[:], ones[:], cnt[:],
                                         start=True, stop=True)
                        nc.vector.tensor_scalar(pred[:], pcnt[:], MED_RANK, None,
                                                op0=ALU.is_lt)
                        nc.vector.tensor_reduce(idx[:], pred[:], axis=AX.X,
                                                op=ALU.add)
                        nc.vector.tensor_scalar(step[:], idx[:], w16[:], None,
                                                op0=ALU.mult)
                        nc.vector.tensor_tensor(lo[:], lo[:], step[:], op=ALU.add)
                        nc.vector.tensor_scalar(w16[:], w16[:], 1.0 / 16.0, None,
                                                op0=ALU.mult)
                    # med = lo + 8*w16  (midpoint of final interval)
                    nc.vector.tensor_scalar(med[:], w16[:], 8.0, lo[:],
                                            op0=ALU.mult, op1=ALU.add)
                    nc.vector.tensor_scalar(rcp_sp[:], rcp_s[:], med[:], None,
                                            op0=ALU.mult)

                # ============ PASS 2: norm + A1T ============
                a1 = sp.tile([H1, R], F32)
                with tc.tile_pool(name="npool", bufs=4) as np_, \
                     tc.tile_pool(name="sqpool", bufs=2) as qp, \
                     tc.tile_pool(name="ps_tr", bufs=3, space="PSUM") as pst_p, \
                     tc.tile_pool(name="ps_a1", bufs=2, space="PSUM") as psa_p:
                    for t in range(NT):
                        xt = xp.tile([128, D], F32, tag="x")
                        nc.sync.dma_start(out=xt[:],
                                          in_=x_d[t * 128:(t + 1) * 128, :])
                        diag = np_.tile([128, 128], F32, tag="diag")
                        nc.vector.tensor_scalar(diag[:], ident[:],
                                                rcp_sp[:, t:t + 1], None,
                                                op0=ALU.mult)
                        psa = psa_p.tile([H1, 128], F32, tag="a1")
                        for g in range(8):
                            pst = pst_p.tile([128, 512], F32, tag="tr")
                            for u in range(4):
                                c = 4 * g + u
                                nc.tensor.matmul(
                                    pst[:, u * 128:(u + 1) * 128],
                                    xt[:, c * 128:(c + 1) * 128], diag[:],
                                    start=True, stop=True)
                            nrm = np_.tile([128, 512], F32, tag="nrm")
                            nc.scalar.activation(
                                nrm[:], pst[:], ACTF.Ln, bias=1.0, scale=1.0,
                                accum_out=nsums[:, t * 8 + g:t * 8 + g + 1])
                            sq = qp.tile([128, 512], F32, tag="sq")
                            nc.vector.scalar_tensor_tensor(
                                sq[:], nrm[:], 1.0, nrm[:],
                                op0=ALU.mult, op1=ALU.mult,
                                accum_out=nsq[:, t * 8 + g:t * 8 + g + 1])
                            for u in range(4):
                                c = 4 * g + u
                                nc.tensor.matmul(
                                    psa[:], wi[:, c, :],
                                    nrm[:, u * 128:(u + 1) * 128],
                                    start=(c == 0), stop=(c == NC_ - 1))
                        nc.vector.tensor_copy(a1[:, t * 128:(t + 1) * 128],
                                              psa[:])

            # ============ BN1 stats (+ global norm var) ============
            with tc.tile_pool(name="bnp", bufs=1) as bn, \
                 tc.tile_pool(name="bn_ps", bufs=2, space="PSUM") as bnps:
                scr = bn.tile([128, R], F32)     # shared square scratch
                ns2 = bn.tile([128, 2], F32)
                nst2 = bn.tile([2, 1], F32)
                sum1 = bn.tile([H1, 1], F32)
                sq1 = bn.tile([H1, 1], F32)
                nc.vector.tensor_reduce(sum1[:], a1[:], axis=AX.X, op=ALU.add)
                nc.vector.scalar_tensor_tensor(scr[0:H1, :], a1[:], 1.0, a1[:],
                                               op0=ALU.mult, op1=ALU.mult,
                                               accum_out=sq1[:])
                nc.vector.tensor_reduce(ns2[:, 0:1], nsums[:], axis=AX.X,
                                        op=ALU.add)
                nc.vector.tensor_reduce(ns2[:, 1:2], nsq[:], axis=AX.X,
                                        op=ALU.add)
                pns = bnps.tile([2, 1], F32, tag="s")
                nc.tensor.matmul(pns[:], ns2[:], ones[:, 0:1],
                                 start=True, stop=True)
                nc.vector.tensor_copy(nst2[:], pns[:])

                ar1_in = dp.tile([2 * H1 + 2], F32)
                ar1_out = dp.tile([2 * H1 + 2], F32, addr_space="Shared")
                nc.sync.dma_start(
                    out=ar1_in[0:H1].rearrange("(p f) -> p f", f=1), in_=sum1[:])
                nc.sync.dma_start(
                    out=ar1_in[H1:2 * H1].rearrange("(p f) -> p f", f=1),
                    in_=sq1[:])
                nc.sync.dma_start(
                    out=ar1_in[2 * H1:2 * H1 + 2].rearrange("(p f) -> p f", f=1),
                    in_=nst2[:])
                nc.gpsimd.collective_compute(
                    "AllReduce", ALU.add,
                    replica_groups=[list(range(N_CORES))],
                    ins=[ar1_in.opt()], outs=[ar1_out.opt()])
                sum1g = bn.tile([H1, 1], F32)
                sq1g = bn.tile([H1, 1], F32)
                nstg = bn.tile([1, 2], F32)
                nc.sync.dma_start(
                    out=sum1g[:], in_=ar1_out[0:H1].rearrange("(p f) -> p f", f=1))
                nc.sync.dma_start(
                    out=sq1g[:],
                    in_=ar1_out[H1:2 * H1].rearrange("(p f) -> p f", f=1))
                nc.sync.dma_start(
                    out=nstg[:],
                    in_=ar1_out[2 * H1:2 * H1 + 2].rearrange("(p f) -> p f", p=1))

                # sigma^2*eps = 1e-5 * (Sn2 - Sn^2/N) / (N-1)
                t1 = bn.tile([1, 1], F32)
                nc.vector.tensor_tensor(t1[:], nstg[:, 0:1], nstg[:, 0:1],
                                        op=ALU.mult)
                nc.vector.tensor_scalar(t1[:], t1[:], 1.0 / N_ELEMS, None,
                                        op0=ALU.mult)
                nc.vector.tensor_tensor(t1[:], nstg[:, 1:2], t1[:],
                                        op=ALU.subtract)
                nc.vector.tensor_scalar(t1[:], t1[:], 1e-5 / (N_ELEMS - 1.0),
                                        None, op0=ALU.mult)
                peps = bnps.tile([H1, 1], F32, tag="s")
                nc.tensor.matmul(peps[:], ones[0:1, 0:H1], t1[:],
                                 start=True, stop=True)

                def bn_affine(pool, sumg, sqg, gt, btt, n, eps_ap=None,
                              eps_imm=None):
                    mu = pool.tile([n, 1], F32, name=f"mu{n}_{len(_dbg)}")
                    var = pool.tile([n, 1], F32, name=f"var{n}_{len(_dbg)}")
                    sc = pool.tile([n, 1], F32, name=f"sc{n}_{len(_dbg)}")
                    bi = pool.tile([n, 1], F32, name=f"bi{n}_{len(_dbg)}")
                    _dbg.append(0)
                    nc.vector.tensor_scalar(mu[:], sumg[:], 1.0 / B, None,
                                            op0=ALU.mult)
                    nc.vector.tensor_scalar(var[:], sqg[:], 1.0 / B, None,
                                            op0=ALU.mult)
                    t = pool.tile([n, 1], F32, name=f"t{n}_{len(_dbg)}")
                    nc.vector.tensor_tensor(t[:], mu[:], mu[:], op=ALU.mult)
                    nc.vector.tensor_tensor(var[:], var[:], t[:],
                                            op=ALU.subtract)
                    if eps_ap is not None:
                        nc.vector.tensor_tensor(var[:], var[:], eps_ap,
                                                op=ALU.add)
                    else:
                        nc.vector.tensor_scalar(var[:], var[:], eps_imm, None,
                                                op0=ALU.add)
                    nc.scalar.sqrt(t[:], var[:])
                    nc.vector.reciprocal(t[:], t[:])
                    nc.vector.tensor_tensor(sc[:], t[:], gt[:], op=ALU.mult)
                    nc.vector.tensor_tensor(t[:], mu[:], sc[:], op=ALU.mult)
                    nc.vector.tensor_tensor(bi[:], btt[:], t[:],
                                            op=ALU.subtract)
                    return sc, bi

                _dbg = []
                sc1, bi1 = bn_affine(bn, sum1g, sq1g, g1t, bt1t, H1,
                                     eps_ap=peps[:])
                h1 = bn.tile([H1, R], F32)
                nc.scalar.activation(h1[:], a1[:], ACTF.Relu, bias=bi1[:],
                                     scale=sc1[:])

                # ============ layer 2 ============
                a2 = bn.tile([H2, R], F32)
                for blk in range(NB):
                    pa2 = bnps.tile([H2, 512], F32, tag="l")
                    nc.tensor.matmul(pa2[:], wenc[:],
                                     h1[:, blk * 512:(blk + 1) * 512],
                                     start=True, stop=True)
                    nc.vector.tensor_copy(a2[:, blk * 512:(blk + 1) * 512],
                                          pa2[:])
                sum2 = bn.tile([H2, 1], F32)
                sq2 = bn.tile([H2, 1], F32)
                nc.vector.tensor_reduce(sum2[:], a2[:], axis=AX.X, op=ALU.add)
                nc.vector.scalar_tensor_tensor(scr[0:H2, :], a2[:], 1.0, a2[:],
                                               op0=ALU.mult, op1=ALU.mult,
                                               accum_out=sq2[:])
                ar2_in = dp.tile([2 * H2], F32)
                ar2_out = dp.tile([2 * H2], F32, addr_space="Shared")
                nc.sync.dma_start(
                    out=ar2_in[0:H2].rearrange("(p f) -> p f", f=1), in_=sum2[:])
                nc.sync.dma_start(
                    out=ar2_in[H2:2 * H2].rearrange("(p f) -> p f", f=1),
                    in_=sq2[:])
                nc.gpsimd.collective_compute(
                    "AllReduce", ALU.add,
                    replica_groups=[list(range(N_CORES))],
                    ins=[ar2_in.opt()], outs=[ar2_out.opt()])
                sum2g = bn.tile([H2, 1], F32)
                sq2g = bn.tile([H2, 1], F32)
                nc.sync.dma_start(
                    out=sum2g[:], in_=ar2_out[0:H2].rearrange("(p f) -> p f", f=1))
                nc.sync.dma_start(
                    out=sq2g[:],
                    in_=ar2_out[H2:2 * H2].rearrange("(p f) -> p f", f=1))
                sc2, bi2 = bn_affine(bn, sum2g, sq2g, g2t, bt2t, H2,
                                     eps_imm=1e-5)
                h2 = bn.tile([H2, R], F32)
                nc.scalar.activation(h2[:], a2[:], ACTF.Relu, bias=bi2[:],
                                     scale=sc2[:])

                # ============ layer 3 ============
                a3 = bn.tile([H1, R], F32)
                for blk in range(NB):
                    pa3 = bnps.tile([H1, 512], F32, tag="l")
                    nc.tensor.matmul(pa3[:], wdec[:],
                                     h2[:, blk * 512:(blk + 1) * 512],
                                     start=True, stop=True)
                    nc.vector.tensor_copy(a3[:, blk * 512:(blk + 1) * 512],
                                          pa3[:])
                sum3 = bn.tile([H1, 1], F32)
                sq3 = bn.tile([H1, 1], F32)
                nc.vector.tensor_reduce(sum3[:], a3[:], axis=AX.X, op=ALU.add)
                nc.vector.scalar_tensor_tensor(scr[0:H1, :], a3[:], 1.0, a3[:],
                                               op0=ALU.mult, op1=ALU.mult,
                                               accum_out=sq3[:])
                ar3_in = dp.tile([2 * H1], F32)
                ar3_out = dp.tile([2 * H1], F32, addr_space="Shared")
                nc.sync.dma_start(
                    out=ar3_in[0:H1].rearrange("(p f) -> p f", f=1), in_=sum3[:])
                nc.sync.dma_start(
                    out=ar3_in[H1:2 * H1].rearrange("(p f) -> p f", f=1),
                    in_=sq3[:])
                nc.gpsimd.collective_compute(
                    "AllReduce", ALU.add,
                    replica_groups=[list(range(N_CORES))],
                    ins=[ar3_in.opt()], outs=[ar3_out.opt()])
                sum3g = bn.tile([H1, 1], F32)
                sq3g = bn.tile([H1, 1], F32)
                nc.sync.dma_start(
                    out=sum3g[:], in_=ar3_out[0:H1].rearrange("(p f) -> p f", f=1))
                nc.sync.dma_start(
                    out=sq3g[:],
                    in_=ar3_out[H1:2 * H1].rearrange("(p f) -> p f", f=1))
                sc3, bi3 = bn_affine(bn, sum3g, sq3g, g3t, bt3t, H1,
                                     eps_imm=1e-5)
                h3e = sp.tile([H1 + 1, R], F32)
                nc.vector.memset(h3e[H1:H1 + 1, :], 1.0)
                nc.scalar.activation(h3e[0:H1, :], a3[:], ACTF.Relu,
                                     bias=bi3[:], scale=sc3[:])

            # ============ heads ============
            funcs = [ACTF.Sigmoid, ACTF.Exp, ACTF.Exp]
            with tc.tile_pool(name="hpool", bufs=6) as hp, \
                 tc.tile_pool(name="hps", bufs=4, space="PSUM") as hps:
                for t in range(NT):
                    for h in range(3):
                        for cc in range(8):
                            ph = hps.tile([128, 512], F32, tag="h")
                            nc.tensor.matmul(
                                ph[:], h3e[:, t * 128:(t + 1) * 128],
                                whe[:, h, cc * 512:(cc + 1) * 512],
                                start=True, stop=True)
                            ot = hp.tile([128, 512], F32, tag="o")
                            nc.scalar.activation(ot[:], ph[:], funcs[h])
                            nc.sync.dma_start(
                                out=out_d[h][t * 128:(t + 1) * 128,
                                             cc * 512:(cc + 1) * 512],
                                in_=ot[:])

    nc.compile()
    return nc


def _consts():
    j15 = np.tile(np.arange(1, 16, dtype=np.float32), (128, 1))
    return {
        "ident": np.eye(128, dtype=np.float32),
        "ones": np.ones((128, 128), dtype=np.float32),
        "j15": j15,
    }


LAST_RESULT = None


def kernel(**inputs):
    global LAST_RESULT
    if "nc" not in _CACHE:
        _CACHE["nc"] = _build()
    nc = _CACHE["nc"]

    np_in = {k: np.asarray(v, dtype=np.float32) for k, v in inputs.items()}
    x = np_in["x"]
    shared = {k: np_in[k] for k in np_in if k != "x"}
    shared.update(_consts())
    in_maps = []
    for c in range(N_CORES):
        m = dict(shared)
        m["x"] = np.ascontiguousarray(x[c * R:(c + 1) * R])
        in_maps.append(m)

    res = run_bass_kernel_spmd(nc, in_maps, core_ids=list(range(N_CORES)))
    LAST_RESULT = res
    pi = np.concatenate([res.results[c]["PI"] for c in range(N_CORES)], axis=0)
    m_ = np.concatenate([res.results[c]["M"] for c in range(N_CORES)], axis=0)
    th = np.concatenate([res.results[c]["TH"] for c in range(N_CORES)], axis=0)
    return (pi, m_, th)


# revision 3
# speedup vs baseline: 1.4161x; 1.4161x over previous
"""Trainium2 Bass kernel for nn_AutoEncoder_51642686767592.

Data-parallel over the batch dim across 8 NeuronCores. Global reductions
(median of row sums, global norm stats, BatchNorm batch stats) run on-device
via collectives (AllGather + 3 small AllReduces).

Math notes (vs reference):
  preprocess: s = x.sum(1); med = lower-median(s); norm = log(x/(s/med) + 1)
  h = (norm - mean)/std(ddof=1)       <- folded into BN1:
  BN1(h@W_in + b_in) == (A - muA) * rsqrt(varA + sigma^2*eps) * g1 + bt1
      where A = norm@W_in (no bias), sigma^2 = global var(norm, ddof=1).
  b_in/b_enc/b_dec and the global mean cancel inside BatchNorm.
  Head biases are applied via a ones-row (K=65) in the head matmuls.

Perf notes:
  - x is transposed on-chip by a PE matmul against diag(med/s): one op yields
    the scaled transpose in PSUM; ACT Ln(psum+1) drains it to SBUF (float32r)
    with a fused per-partition sum (-> global norm mean).
  - All dense matmuls vs weights run in float32r (1 cycle/row at N>=256,
    4x faster than fp32's hi/lo 2-pass; ~1e-4 component rounding).
  - Heads iterate head-major so ACT keeps one LUT loaded per function.
"""
import numpy as np

import concourse.bacc as bacc
import concourse.mybir as mybir
import concourse.tile as tile
from concourse.bass_utils import run_bass_kernel_spmd

F32 = mybir.dt.float32
F32R = mybir.dt.float32r
ALU = mybir.AluOpType
ACTF = mybir.ActivationFunctionType
AX = mybir.AxisListType

N_CORES = 8
B, D = 16384, 4096
H1, H2 = 64, 32
R = B // N_CORES          # rows per core = 2048
NT = R // 128             # 128-row tiles per core = 16
NBLK = R // 256           # 256-row blocks per core = 8
NC_ = D // 128            # d chunks = 32
N_ELEMS = float(B * D)
MED_RANK = 8192.0         # count(s <= t) >= 8192  <=>  t >= lower median
BIS_ITERS = 7

_CACHE = {}


def _build():
    nc = bacc.Bacc("TRN2", target_bir_lowering=False, debug=False,
                   num_devices=N_CORES)
    RG = [list(range(N_CORES))]

    x_d = nc.dram_tensor("x", [R, D], F32, kind="ExternalInput")
    win_d = nc.dram_tensor("W_in", [D, H1], F32, kind="ExternalInput")
    wenc_d = nc.dram_tensor("W_enc", [H1, H2], F32, kind="ExternalInput")
    wdec_d = nc.dram_tensor("W_dec", [H2, H1], F32, kind="ExternalInput")
    wh_d = [nc.dram_tensor(n, [H1, D], F32, kind="ExternalInput")
            for n in ("W_pi", "W_m", "W_th")]
    bh_d = [nc.dram_tensor(n, [D], F32, kind="ExternalInput")
            for n in ("b_pi", "b_m", "b_th")]
    g_d = [nc.dram_tensor(n, [sz], F32, kind="ExternalInput")
           for n, sz in (("g1", H1), ("bt1", H1), ("g2", H2), ("bt2", H2),
                         ("g3", H1), ("bt3", H1))]
    ident_d = nc.dram_tensor("ident", [128, 128], F32, kind="ExternalInput")
    ones_d = nc.dram_tensor("ones", [128, 128], F32, kind="ExternalInput")
    j15_d = nc.dram_tensor("j15", [128, 15], F32, kind="ExternalInput")

    out_d = [nc.dram_tensor(n, [R, D], F32, kind="ExternalOutput")
             for n in ("PI", "M", "TH")]

    with tile.TileContext(nc) as tc:
        with tc.tile_pool(name="wpool", bufs=1) as wp, \
             tc.tile_pool(name="spool", bufs=1) as sp, \
             tc.tile_pool(name="dram", bufs=1, space="DRAM") as dp:

            # ---- warm up the collectives stack (overlaps pass 1) ----
            cw_in = dp.tile([128], F32)
            cw_out = dp.tile([128], F32, addr_space="Shared")
            cw_t = sp.tile([1, 128], F32)
            nc.vector.memset(cw_t[:], 0.0)
            nc.sync.dma_start(out=cw_in[:].rearrange("(p f) -> p f", p=1),
                              in_=cw_t[:])
            nc.gpsimd.collective_compute(
                "AllReduce", ALU.add, replica_groups=RG,
                ins=[cw_in.opt()], outs=[cw_out.opt()])
            cw2_in = dp.tile([B], F32)
            cw2_out = dp.tile([B], F32, addr_space="Shared")
            nc.sync.dma_start(out=cw2_in[0:128].rearrange("(p f) -> p f", p=1),
                              in_=cw_t[:])
            nc.gpsimd.collective_compute(
                "AllGather", ALU.bypass, replica_groups=RG,
                ins=[cw2_in[0:R].opt()], outs=[cw2_out.opt()])

            # ---- constants / weights resident in SBUF ----
            ident = wp.tile([128, 128], F32)
            nc.sync.dma_start(out=ident[:], in_=ident_d[:])
            ones = wp.tile([128, 128], F32)
            nc.sync.dma_start(out=ones[:], in_=ones_d[:])
            j15 = wp.tile([128, 15], F32)
            nc.sync.dma_start(out=j15[:], in_=j15_d[:])
            wi = wp.tile([128, NC_, H1], F32R)   # W_in as [p, chunk, 64], f32r
            nc.gpsimd.dma_start(out=wi[:],
                                in_=win_d[:].rearrange("(c p) k -> p c k", p=128))
            wenc = wp.tile([H1, H2], F32R)
            nc.gpsimd.dma_start(out=wenc[:], in_=wenc_d[:])
            wdec = wp.tile([H2, H1], F32R)
            nc.gpsimd.dma_start(out=wdec[:], in_=wdec_d[:])
            whe = wp.tile([H1 + 1, 3, D], F32R)  # [W_head; b_head], f32r
            for h in range(3):
                nc.gpsimd.dma_start(out=whe[0:H1, h, :], in_=wh_d[h][:])
                nc.gpsimd.dma_start(out=whe[H1:H1 + 1, h, :],
                                    in_=bh_d[h][:].rearrange("(p f) -> p f", p=1))
            gbt = []
            for t_d in g_d:
                sz = t_d.shape[0]
                tt = wp.tile([sz, 1], F32, name=f"c_{t_d.name}")
                nc.sync.dma_start(out=tt[:],
                                  in_=t_d[:].rearrange("(p f) -> p f", f=1))
                gbt.append(tt)
            g1t, bt1t, g2t, bt2t, g3t, bt3t = gbt

            svals = sp.tile([128, NT], F32)
            rcp_s = sp.tile([128, NT], F32)
            rcp_sp = sp.tile([128, NT], F32)
            s_all = sp.tile([128, 128], F32)
            nsums = sp.tile([128, NC_ * NBLK], F32)
            nsq = sp.tile([128, NC_ * NBLK], F32)

            # ============ PASS 1: row sums ============
            with tc.tile_pool(name="xpool", bufs=4) as xp:
                for t in range(NT):
                    xt = xp.tile([128, D], F32, tag="x")
                    nc.sync.dma_start(out=xt[:], in_=x_d[t * 128:(t + 1) * 128, :])
                    nc.vector.tensor_reduce(svals[:, t:t + 1], xt[:],
                                            axis=AX.X, op=ALU.add)
                nc.vector.reciprocal(rcp_s[:], svals[:])

                sb_in = dp.tile([R], F32)
                sb_out = dp.tile([B], F32, addr_space="Shared")
                nc.sync.dma_start(out=sb_in[:].rearrange("(p t) -> p t", p=128),
                                  in_=svals[:])
                nc.gpsimd.collective_compute(
                    "AllGather", ALU.bypass, replica_groups=RG,
                    ins=[sb_in.opt()], outs=[sb_out.opt()])
                nc.sync.dma_start(out=s_all[:],
                                  in_=sb_out[:].rearrange("(p f) -> p f", p=128))

                # ============ median: 16-ary search ============
                with tc.tile_pool(name="bis", bufs=1) as bp, \
                     tc.tile_pool(name="bps", bufs=1, space="PSUM") as bps:
                    lo = bp.tile([128, 1], F32)
                    w16 = bp.tile([128, 1], F32)
                    nc.vector.memset(lo[:], 0.0)
                    nc.vector.memset(w16[:], float(D) / 16.0)
                    thr = bp.tile([128, 15], F32)
                    cnt = bp.tile([128, 15], F32)
                    cscr = bp.tile([128, 2, 128], F32)
                    pred = bp.tile([128, 15], F32)
                    idx = bp.tile([128, 1], F32)
                    step = bp.tile([128, 1], F32)
                    med = bp.tile([128, 1], F32)
                    for it in range(BIS_ITERS):
                        nc.vector.tensor_scalar(thr[:], j15[:], w16[:], lo[:],
                                                op0=ALU.mult, op1=ALU.add)
                        for j in range(15):
                            nc.vector.tensor_scalar(
                                cscr[:, j % 2, :], s_all[:], thr[:, j:j + 1],
                                None, op0=ALU.is_le, op1=ALU.add,
                                accum_out=cnt[:, j:j + 1])
                        pcnt = bps.tile([128, 15], F32, tag="pcnt")
                        nc.tensor.matmul(pcnt[:], ones[:], cnt[:],
                                         start=True, stop=True)
                        nc.vector.tensor_scalar(pred[:], pcnt[:], MED_RANK, None,
                                                op0=ALU.is_lt)
                        nc.vector.tensor_reduce(idx[:], pred[:], axis=AX.X,
                                                op=ALU.add)
                        nc.vector.tensor_scalar(step[:], idx[:], w16[:], None,
                                                op0=ALU.mult)
                        nc.vector.tensor_tensor(lo[:], lo[:], step[:], op=ALU.add)
                        nc.vector.tensor_scalar(w16[:], w16[:], 1.0 / 16.0, None,
                                                op0=ALU.mult)
                    nc.vector.tensor_scalar(med[:], w16[:], 8.0, lo[:],
                                            op0=ALU.mult, op1=ALU.add)
                    nc.vector.tensor_scalar(rcp_sp[:], rcp_s[:], med[:], None,
                                            op0=ALU.mult)

                # ============ PASS 2: norm + A1T (256-row blocks) ============
                a1 = sp.tile([H1, R], F32)
                with tc.tile_pool(name="npool", bufs=4) as np_, \
                     tc.tile_pool(name="sqpool", bufs=2) as qp, \
                     tc.tile_pool(name="ps_tr", bufs=3, space="PSUM") as pst_p, \
                     tc.tile_pool(name="ps_a1", bufs=2, space="PSUM") as psa_p:
                    for blk in range(NBLK):
                        xts, diags = [], []
                        for u in range(2):
                            t = 2 * blk + u
                            xt = xp.tile([128, D], F32, tag="x")
                            nc.sync.dma_start(
                                out=xt[:], in_=x_d[t * 128:(t + 1) * 128, :])
                            dg = np_.tile([128, 128], F32, tag="diag")
                            nc.vector.tensor_scalar(dg[:], ident[:],
                                                    rcp_sp[:, t:t + 1], None,
                                                    op0=ALU.mult)
                            xts.append(xt)
                            diags.append(dg)
                        psa = psa_p.tile([H1, 256], F32, tag="a1")
                        for c in range(NC_):
                            pst = pst_p.tile([128, 256], F32, tag="tr")
                            for u in range(2):
                                nc.tensor.matmul(
                                    pst[:, u * 128:(u + 1) * 128],
                                    xts[u][:, c * 128:(c + 1) * 128],
                                    diags[u][:], start=True, stop=True)
                            nrm = np_.tile([128, 256], F32R, tag="nrm")
                            col = blk * NC_ + c
                            nc.scalar.activation(
                                nrm[:], pst[:], ACTF.Ln, bias=1.0, scale=1.0,
                                accum_out=nsums[:, col:col + 1])
                            sq = qp.tile([128, 256], F32, tag="sq")
                            nc.vector.scalar_tensor_tensor(
                                sq[:], nrm[:], 1.0, nrm[:],
                                op0=ALU.mult, op1=ALU.mult,
                                accum_out=nsq[:, col:col + 1])
                            nc.tensor.matmul(psa[:], wi[:, c, :], nrm[:],
                                             start=(c == 0), stop=(c == NC_ - 1))
                        nc.vector.tensor_copy(a1[:, blk * 256:(blk + 1) * 256],
                                              psa[:])

            # ============ BN1 stats (+ global norm var) ============
            with tc.tile_pool(name="bnp", bufs=1) as bn, \
                 tc.tile_pool(name="bn_ps", bufs=2, space="PSUM") as bnps:
                scr = bn.tile([128, 512], F32)
                ns2 = bn.tile([128, 2], F32)
                nst2 = bn.tile([2, 1], F32)

                def colstats(src, n, sums, sqs):
                    nc.vector.tensor_reduce(sums[:], src[:], axis=AX.X,
                                            op=ALU.add)
                    part = bn.tile([n, 4], F32, name=f"part{n}_{len(_dbg)}")
                    _dbg.append(0)
                    for q in range(4):
                        nc.vector.scalar_tensor_tensor(
                            scr[0:n, :], src[:, q * 512:(q + 1) * 512], 1.0,
                            src[:, q * 512:(q + 1) * 512],
                            op0=ALU.mult, op1=ALU.mult,
                            accum_out=part[:, q:q + 1])
                    nc.vector.tensor_reduce(sqs[:], part[:], axis=AX.X,
                                            op=ALU.add)

                _dbg = []
                sum1 = bn.tile([H1, 1], F32)
                sq1 = bn.tile([H1, 1], F32)
                colstats(a1, H1, sum1, sq1)
                nc.vector.tensor_reduce(ns2[:, 0:1], nsums[:], axis=AX.X,
                                        op=ALU.add)
                nc.vector.tensor_reduce(ns2[:, 1:2], nsq[:], axis=AX.X,
                                        op=ALU.add)
                pns = bnps.tile([2, 1], F32, tag="s")
                nc.tensor.matmul(pns[:], ns2[:], ones[:, 0:1],
                                 start=True, stop=True)
                nc.vector.tensor_copy(nst2[:], pns[:])

                ar1_in = dp.tile([2 * H1 + 2], F32)
                ar1_out = dp.tile([2 * H1 + 2], F32, addr_space="Shared")
                nc.sync.dma_start(
                    out=ar1_in[0:H1].rearrange("(p f) -> p f", f=1), in_=sum1[:])
                nc.sync.dma_start(
                    out=ar1_in[H1:2 * H1].rearrange("(p f) -> p f", f=1),
                    in_=sq1[:])
                nc.sync.dma_start(
                    out=ar1_in[2 * H1:2 * H1 + 2].rearrange("(p f) -> p f", f=1),
                    in_=nst2[:])
                nc.gpsimd.collective_compute(
                    "AllReduce", ALU.add, replica_groups=RG,
                    ins=[ar1_in.opt()], outs=[ar1_out.opt()])
                sum1g = bn.tile([H1, 1], F32)
                sq1g = bn.tile([H1, 1], F32)
                nstg = bn.tile([1, 2], F32)
                nc.sync.dma_start(
                    out=sum1g[:], in_=ar1_out[0:H1].rearrange("(p f) -> p f", f=1))
                nc.sync.dma_start(
                    out=sq1g[:],
                    in_=ar1_out[H1:2 * H1].rearrange("(p f) -> p f", f=1))
                nc.sync.dma_start(
                    out=nstg[:],
                    in_=ar1_out[2 * H1:2 * H1 + 2].rearrange("(p f) -> p f", p=1))

                t1 = bn.tile([1, 1], F32)
                nc.vector.tensor_tensor(t1[:], nstg[:, 0:1], nstg[:, 0:1],
                                        op=ALU.mult)
                nc.vector.tensor_scalar(t1[:], t1[:], 1.0 / N_ELEMS, None,
                                        op0=ALU.mult)
                nc.vector.tensor_tensor(t1[:], nstg[:, 1:2], t1[:],
                                        op=ALU.subtract)
                nc.vector.tensor_scalar(t1[:], t1[:], 1e-5 / (N_ELEMS - 1.0),
                                        None, op0=ALU.mult)
                peps = bnps.tile([H1, 1], F32, tag="s")
                nc.tensor.matmul(peps[:], ones[0:1, 0:H1], t1[:],
                                 start=True, stop=True)

                def bn_affine(sumg, sqg, gt, btt, n, eps_ap=None, eps_imm=None):
                    k = len(_dbg)
                    _dbg.append(0)
                    mu = bn.tile([n, 1], F32, name=f"mu_{k}")
                    var = bn.tile([n, 1], F32, name=f"var_{k}")
                    sc = bn.tile([n, 1], F32, name=f"sc_{k}")
                    bi = bn.tile([n, 1], F32, name=f"bi_{k}")
                    t = bn.tile([n, 1], F32, name=f"tt_{k}")
                    nc.vector.tensor_scalar(mu[:], sumg[:], 1.0 / B, None,
                                            op0=ALU.mult)
                    nc.vector.tensor_tensor(t[:], mu[:], mu[:], op=ALU.mult)
                    nc.vector.tensor_scalar(var[:], sqg[:], 1.0 / B, t[:],
                                            op0=ALU.mult, op1=ALU.subtract)
                    if eps_ap is not None:
                        nc.vector.tensor_tensor(var[:], var[:], eps_ap,
                                                op=ALU.add)
                    else:
                        nc.vector.tensor_scalar(var[:], var[:], eps_imm, None,
                                                op0=ALU.add)
                    nc.scalar.sqrt(t[:], var[:])
                    nc.vector.reciprocal(t[:], t[:])
                    nc.vector.tensor_tensor(sc[:], t[:], gt[:], op=ALU.mult)
                    nc.vector.tensor_tensor(t[:], mu[:], sc[:], op=ALU.mult)
                    nc.vector.tensor_tensor(bi[:], btt[:], t[:],
                                            op=ALU.subtract)
                    return sc, bi

                sc1, bi1 = bn_affine(sum1g, sq1g, g1t, bt1t, H1, eps_ap=peps[:])
                h1 = bn.tile([H1, R], F32R)
                nc.scalar.activation(h1[:], a1[:], ACTF.Relu, bias=bi1[:],
                                     scale=sc1[:])

                # ============ layer 2 ============
                a2 = bn.tile([H2, R], F32)
                for blk in range(4):
                    pa2 = bnps.tile([H2, 512], F32, tag="l")
                    nc.tensor.matmul(pa2[:], wenc[:],
                                     h1[:, blk * 512:(blk + 1) * 512],
                                     start=True, stop=True)
                    nc.vector.tensor_copy(a2[:, blk * 512:(blk + 1) * 512],
                                          pa2[:])
                sum2 = bn.tile([H2, 1], F32)
                sq2 = bn.tile([H2, 1], F32)
                colstats(a2, H2, sum2, sq2)
                ar2_in = dp.tile([2 * H2], F32)
                ar2_out = dp.tile([2 * H2], F32, addr_space="Shared")
                nc.sync.dma_start(
                    out=ar2_in[0:H2].rearrange("(p f) -> p f", f=1), in_=sum2[:])
                nc.sync.dma_start(
                    out=ar2_in[H2:2 * H2].rearrange("(p f) -> p f", f=1),
                    in_=sq2[:])
                nc.gpsimd.collective_compute(
                    "AllReduce", ALU.add, replica_groups=RG,
                    ins=[ar2_in.opt()], outs=[ar2_out.opt()])
                sum2g = bn.tile([H2, 1], F32)
                sq2g = bn.tile([H2, 1], F32)
                nc.sync.dma_start(
                    out=sum2g[:], in_=ar2_out[0:H2].rearrange("(p f) -> p f", f=1))
                nc.sync.dma_start(
                    out=sq2g[:],
                    in_=ar2_out[H2:2 * H2].rearrange("(p f) -> p f", f=1))
                sc2, bi2 = bn_affine(sum2g, sq2g, g2t, bt2t, H2, eps_imm=1e-5)
                h2 = bn.tile([H2, R], F32R)
                nc.scalar.activation(h2[:], a2[:], ACTF.Relu, bias=bi2[:],
                                     scale=sc2[:])

                # ============ layer 3 ============
                a3 = bn.tile([H1, R], F32)
                for blk in range(4):
                    pa3 = bnps.tile([H1, 512], F32, tag="l")
                    nc.tensor.matmul(pa3[:], wdec[:],
                                     h2[:, blk * 512:(blk + 1) * 512],
                                     start=True, stop=True)
                    nc.vector.tensor_copy(a3[:, blk * 512:(blk + 1) * 512],
                                          pa3[:])
                sum3 = bn.tile([H1, 1], F32)
                sq3 = bn.tile([H1, 1], F32)
                colstats(a3, H1, sum3, sq3)
                ar3_in = dp.tile([2 * H1], F32)
                ar3_out = dp.tile([2 * H1], F32, addr_space="Shared")
                nc.sync.dma_start(
                    out=ar3_in[0:H1].rearrange("(p f) -> p f", f=1), in_=sum3[:])
                nc.sync.dma_start(
                    out=ar3_in[H1:2 * H1].rearrange("(p f) -> p f", f=1),
                    in_=sq3[:])
                nc.gpsimd.collective_compute(
                    "AllReduce", ALU.add, replica_groups=RG,
                    ins=[ar3_in.opt()], outs=[ar3_out.opt()])
                sum3g = bn.tile([H1, 1], F32)
                sq3g = bn.tile([H1, 1], F32)
                nc.sync.dma_start(
                    out=sum3g[:], in_=ar3_out[0:H1].rearrange("(p f) -> p f", f=1))
                nc.sync.dma_start(
                    out=sq3g[:],
                    in_=ar3_out[H1:2 * H1].rearrange("(p f) -> p f", f=1))
                sc3, bi3 = bn_affine(sum3g, sq3g, g3t, bt3t, H1, eps_imm=1e-5)
                h3e = sp.tile([H1 + 1, R], F32R)
                nc.vector.memset(h3e[H1:H1 + 1, :].bitcast(F32), 1.0)
                nc.scalar.activation(h3e[0:H1, :], a3[:], ACTF.Relu,
                                     bias=bi3[:], scale=sc3[:])

            # ============ heads (head-major: one ACT LUT per head) ============
            funcs = [ACTF.Sigmoid, ACTF.Exp, ACTF.Exp]
            with tc.tile_pool(name="hpool", bufs=3) as hp, \
                 tc.tile_pool(name="hps", bufs=4, space="PSUM") as hps:
                for h in range(3):
                    for t in range(NT):
                        for cc2 in range(4):
                            ot = hp.tile([128, 1024], F32, tag="o")
                            for half in range(2):
                                cc = 2 * cc2 + half
                                ph = hps.tile([128, 512], F32, tag="h")
                                nc.tensor.matmul(
                                    ph[:], h3e[:, t * 128:(t + 1) * 128],
                                    whe[:, h, cc * 512:(cc + 1) * 512],
                                    start=True, stop=True)
                                nc.scalar.activation(
                                    ot[:, half * 512:(half + 1) * 512],
                                    ph[:], funcs[h])
                            nc.sync.dma_start(
                                out=out_d[h][t * 128:(t + 1) * 128,
                                             cc2 * 1024:(cc2 + 1) * 1024],
                                in_=ot[:])

    nc.compile()
    return nc


def _consts():
    return {
        "ident": np.eye(128, dtype=np.float32),
        "ones": np.ones((128, 128), dtype=np.float32),
        "j15": np.tile(np.arange(1, 16, dtype=np.float32), (128, 1)),
    }


LAST_RESULT = None


def kernel(**inputs):
    global LAST_RESULT
    if "nc" not in _CACHE:
        _CACHE["nc"] = _build()
    nc = _CACHE["nc"]

    np_in = {k: np.asarray(v, dtype=np.float32) for k, v in inputs.items()}
    x = np_in["x"]
    shared = {k: np_in[k] for k in np_in if k != "x"}
    shared.update(_consts())
    in_maps = []
    for c in range(N_CORES):
        m = dict(shared)
        m["x"] = np.ascontiguousarray(x[c * R:(c + 1) * R])
        in_maps.append(m)

    res = run_bass_kernel_spmd(nc, in_maps, core_ids=list(range(N_CORES)))
    LAST_RESULT = res
    pi = np.concatenate([res.results[c]["PI"] for c in range(N_CORES)], axis=0)
    m_ = np.concatenate([res.results[c]["M"] for c in range(N_CORES)], axis=0)
    th = np.concatenate([res.results[c]["TH"] for c in range(N_CORES)], axis=0)
    return (pi, m_, th)


# revision 14
# speedup vs baseline: 1.6443x; 1.1612x over previous
"""Trainium2 Bass kernel for nn_AutoEncoder_51642686767592.

Data-parallel over the batch dim across 8 NeuronCores. Global reductions
(median of row sums, global norm stats, BatchNorm batch stats) run on-device
via collectives (AllGather + 3 small AllReduces).

Math notes (vs reference):
  preprocess: s = x.sum(1); med = lower-median(s); norm = log(x/(s/med) + 1)
  h = (norm - mean)/std(ddof=1)       <- folded into BN1:
  BN1(h@W_in + b_in) == (A - muA) * rsqrt(varA + sigma^2*eps) * g1 + bt1
      where A = norm@W_in (no bias), sigma^2 = global var(norm, ddof=1).
  b_in/b_enc/b_dec and the global mean cancel inside BatchNorm.
  Head biases are applied via a ones-row (K=65) in the head matmuls.

Perf notes:
  - x is transposed on-chip by a PE matmul against diag(med/s): one op yields
    the scaled transpose in PSUM; ACT Ln(psum+1) drains it to SBUF (float32r)
    with a fused per-partition sum (-> global norm mean).
  - All dense matmuls vs weights run in float32r (1 cycle/row at N>=256,
    4x faster than fp32's hi/lo 2-pass; ~1e-4 component rounding).
  - Heads iterate head-major so ACT keeps one LUT loaded per function.
"""
import numpy as np

import concourse.bacc as bacc
import concourse.mybir as mybir
import concourse.tile as tile
from concourse.bass_utils import run_bass_kernel_spmd

F32 = mybir.dt.float32
F32R = mybir.dt.float32r
ALU = mybir.AluOpType
ACTF = mybir.ActivationFunctionType
AX = mybir.AxisListType

N_CORES = 8
B, D = 16384, 4096
H1, H2 = 64, 32
R = B // N_CORES          # rows per core = 2048
NT = R // 128             # 128-row tiles per core = 16
NBLK = R // 256           # 256-row blocks per core = 8
NC_ = D // 128            # d chunks = 32
N_ELEMS = float(B * D)
MED_RANK = 8192.0         # count(s <= t) >= 8192  <=>  t >= lower median
BIS_ITERS = 5             # 16-ary: final width 4096/16^5 ~ 3.9e-3 (rel 2e-6)
MOM_STRIDE = 8            # sample global norm moments on every 8th chunk

_CACHE = {}


def _build():
    nc = bacc.Bacc("TRN2", target_bir_lowering=False, debug=False,
                   num_devices=N_CORES)
    RG = [list(range(N_CORES))]

    x_d = nc.dram_tensor("x", [R, D], F32, kind="ExternalInput")
    win_d = nc.dram_tensor("W_in", [D, H1], F32, kind="ExternalInput")
    wenc_d = nc.dram_tensor("W_enc", [H1, H2], F32, kind="ExternalInput")
    wdec_d = nc.dram_tensor("W_dec", [H2, H1], F32, kind="ExternalInput")
    wh_d = [nc.dram_tensor(n, [H1, D], F32, kind="ExternalInput")
            for n in ("W_pi", "W_m", "W_th")]
    bh_d = [nc.dram_tensor(n, [D], F32, kind="ExternalInput")
            for n in ("b_pi", "b_m", "b_th")]
    g_d = [nc.dram_tensor(n, [sz], F32, kind="ExternalInput")
           for n, sz in (("g1", H1), ("bt1", H1), ("g2", H2), ("bt2", H2),
                         ("g3", H1), ("bt3", H1))]
    ident_d = nc.dram_tensor("ident", [128, 128], F32, kind="ExternalInput")
    ones_d = nc.dram_tensor("ones", [128, 128], F32, kind="ExternalInput")
    j15_d = nc.dram_tensor("j15", [128, 15], F32, kind="ExternalInput")

    out_d = [nc.dram_tensor(n, [R, D], F32, kind="ExternalOutput")
             for n in ("PI", "M", "TH")]

    with tile.TileContext(nc) as tc:
        with tc.tile_pool(name="wpool", bufs=1) as wp, \
             tc.tile_pool(name="spool", bufs=1) as sp, \
             tc.tile_pool(name="dram", bufs=1, space="DRAM") as dp:

            # ---- warm up the collectives stack (overlaps pass 1) ----
            cw_in = dp.tile([128], F32)
            cw_out = dp.tile([128], F32, addr_space="Shared")
            cw_t = sp.tile([1, 128], F32)
            nc.vector.memset(cw_t[:], 0.0)
            nc.sync.dma_start(out=cw_in[:].rearrange("(p f) -> p f", p=1),
                              in_=cw_t[:])
            nc.gpsimd.collective_compute(
                "AllReduce", ALU.add, replica_groups=RG,
                ins=[cw_in.opt()], outs=[cw_out.opt()])
            cw2_in = dp.tile([B], F32)
            cw2_out = dp.tile([B], F32, addr_space="Shared")
            nc.sync.dma_start(out=cw2_in[0:128].rearrange("(p f) -> p f", p=1),
                              in_=cw_t[:])
            nc.gpsimd.collective_compute(
                "AllGather", ALU.bypass, replica_groups=RG,
                ins=[cw2_in[0:R].opt()], outs=[cw2_out.opt()])

            # ---- constants / weights resident in SBUF ----
            ident = wp.tile([128, 128], F32)
            nc.sync.dma_start(out=ident[:], in_=ident_d[:])
            ones = wp.tile([128, 128], F32)
            nc.sync.dma_start(out=ones[:], in_=ones_d[:])
            j15 = wp.tile([128, 15], F32)
            nc.sync.dma_start(out=j15[:], in_=j15_d[:])
            wi = wp.tile([128, NC_, H1], F32R)   # W_in as [p, chunk, 64], f32r
            nc.gpsimd.dma_start(out=wi[:],
                                in_=win_d[:].rearrange("(c p) k -> p c k", p=128))
            wenc = wp.tile([H1, H2], F32R)
            nc.gpsimd.dma_start(out=wenc[:], in_=wenc_d[:])
            wdec = wp.tile([H2, H1], F32R)
            nc.gpsimd.dma_start(out=wdec[:], in_=wdec_d[:])
            whe = wp.tile([H1 + 1, 3, D], F32R)  # [W_head; b_head], f32r
            for h in range(3):
                nc.gpsimd.dma_start(out=whe[0:H1, h, :], in_=wh_d[h][:])
                nc.gpsimd.dma_start(out=whe[H1:H1 + 1, h, :],
                                    in_=bh_d[h][:].rearrange("(p f) -> p f", p=1))
            gbt = []
            for t_d in g_d:
                sz = t_d.shape[0]
                tt = wp.tile([sz, 1], F32, name=f"c_{t_d.name}")
                nc.sync.dma_start(out=tt[:],
                                  in_=t_d[:].rearrange("(p f) -> p f", f=1))
                gbt.append(tt)
            g1t, bt1t, g2t, bt2t, g3t, bt3t = gbt

            svals = sp.tile([128, NT], F32)
            rcp_s = sp.tile([128, NT], F32)
            rcp_sp = sp.tile([128, NT], F32)
            s_all = sp.tile([128, 128], F32)
            n_mom = (NC_ // MOM_STRIDE) * NBLK
            nsums = sp.tile([128, n_mom], F32)
            nsq = sp.tile([128, n_mom], F32)

            # ============ PASS 1: row sums ============
            with tc.tile_pool(name="xpool", bufs=4) as xp:
                for t in range(NT):
                    xt = xp.tile([128, D], F32, tag="x")
                    nc.sync.dma_start(out=xt[:], in_=x_d[t * 128:(t + 1) * 128, :])
                    nc.vector.tensor_reduce(svals[:, t:t + 1], xt[:],
                                            axis=AX.X, op=ALU.add)
                nc.vector.reciprocal(rcp_s[:], svals[:])

                sb_in = dp.tile([R], F32)
                sb_out = dp.tile([B], F32, addr_space="Shared")
                nc.sync.dma_start(out=sb_in[:].rearrange("(p t) -> p t", p=128),
                                  in_=svals[:])
                nc.gpsimd.collective_compute(
                    "AllGather", ALU.bypass, replica_groups=RG,
                    ins=[sb_in.opt()], outs=[sb_out.opt()])
                nc.sync.dma_start(out=s_all[:],
                                  in_=sb_out[:].rearrange("(p f) -> p f", p=128))

                # ============ median: 16-ary search ============
                with tc.tile_pool(name="bis", bufs=1) as bp, \
                     tc.tile_pool(name="bps", bufs=1, space="PSUM") as bps:
                    lo = bp.tile([128, 1], F32)
                    w16 = bp.tile([128, 1], F32)
                    nc.vector.memset(lo[:], 0.0)
                    nc.vector.memset(w16[:], float(D) / 16.0)
                    thr = bp.tile([128, 15], F32)
                    cnt = bp.tile([128, 15], F32)
                    cscr = bp.tile([128, 2, 128], F32)
                    pred = bp.tile([128, 15], F32)
                    idx = bp.tile([128, 1], F32)
                    step = bp.tile([128, 1], F32)
                    med = bp.tile([128, 1], F32)
                    for it in range(BIS_ITERS):
                        nc.vector.tensor_scalar(thr[:], j15[:], w16[:], lo[:],
                                                op0=ALU.mult, op1=ALU.add)
                        for j in range(15):
                            nc.vector.tensor_scalar(
                                cscr[:, j % 2, :], s_all[:], thr[:, j:j + 1],
                                None, op0=ALU.is_le, op1=ALU.add,
                                accum_out=cnt[:, j:j + 1])
                        pcnt = bps.tile([128, 15], F32, tag="pcnt")
                        nc.tensor.matmul(pcnt[:], ones[:], cnt[:],
                                         start=True, stop=True)
                        nc.vector.tensor_scalar(pred[:], pcnt[:], MED_RANK, None,
                                                op0=ALU.is_lt)
                        nc.vector.tensor_reduce(idx[:], pred[:], axis=AX.X,
                                                op=ALU.add)
                        nc.vector.tensor_scalar(step[:], idx[:], w16[:], None,
                                                op0=ALU.mult)
                        nc.vector.tensor_tensor(lo[:], lo[:], step[:], op=ALU.add)
                        nc.vector.tensor_scalar(w16[:], w16[:], 1.0 / 16.0, None,
                                                op0=ALU.mult)
                    nc.vector.tensor_scalar(med[:], w16[:], 8.0, lo[:],
                                            op0=ALU.mult, op1=ALU.add)
                    nc.vector.tensor_scalar(rcp_sp[:], rcp_s[:], med[:], None,
                                            op0=ALU.mult)

                # ============ PASS 2: norm + A1T (256-row blocks) ============
                a1 = sp.tile([H1, R], F32)
                with tc.tile_pool(name="npool", bufs=4) as np_, \
                     tc.tile_pool(name="sqpool", bufs=2) as qp, \
                     tc.tile_pool(name="ps_tr", bufs=3, space="PSUM") as pst_p, \
                     tc.tile_pool(name="ps_a1", bufs=2, space="PSUM") as psa_p:
                    for blk in range(NBLK):
                        xts, diags = [], []
                        for u in range(2):
                            t = 2 * blk + u
                            xt = xp.tile([128, D], F32, tag="x")
                            nc.sync.dma_start(
                                out=xt[:], in_=x_d[t * 128:(t + 1) * 128, :])
                            dg = np_.tile([128, 128], F32, tag="diag")
                            nc.vector.tensor_scalar(dg[:], ident[:],
                                                    rcp_sp[:, t:t + 1], None,
                                                    op0=ALU.mult)
                            xts.append(xt)
                            diags.append(dg)
                        psa = psa_p.tile([H1, 256], F32, tag="a1")
                        for c in range(NC_):
                            pst = pst_p.tile([128, 256], F32, tag="tr")
                            for u in range(2):
                                nc.tensor.matmul(
                                    pst[:, u * 128:(u + 1) * 128],
                                    xts[u][:, c * 128:(c + 1) * 128],
                                    diags[u][:], start=True, stop=True)
                            nrm = np_.tile([128, 256], F32R, tag="nrm")
                            if c % MOM_STRIDE == 0:
                                col = blk * (NC_ // MOM_STRIDE) + c // MOM_STRIDE
                                nc.scalar.activation(
                                    nrm[:], pst[:], ACTF.Ln, bias=1.0, scale=1.0,
                                    accum_out=nsums[:, col:col + 1])
                                sq = qp.tile([128, 256], F32, tag="sq")
                                nc.vector.scalar_tensor_tensor(
                                    sq[:], nrm[:], 1.0, nrm[:],
                                    op0=ALU.mult, op1=ALU.mult,
                                    accum_out=nsq[:, col:col + 1])
                            else:
                                nc.scalar.activation(
                                    nrm[:], pst[:], ACTF.Ln, bias=1.0, scale=1.0)
                            nc.tensor.matmul(psa[:], wi[:, c, :], nrm[:],
                                             start=(c == 0), stop=(c == NC_ - 1))
                        nc.vector.tensor_copy(a1[:, blk * 256:(blk + 1) * 256],
                                              psa[:])

            # ============ BN1 stats (+ global norm var) ============
            with tc.tile_pool(name="bnp", bufs=1) as bn, \
                 tc.tile_pool(name="bn_ps", bufs=2, space="PSUM") as bnps:
                scr = bn.tile([128, 512], F32)
                ns2 = bn.tile([128, 2], F32)
                nst2 = bn.tile([2, 1], F32)

                def colstats(src, n, st):
                    nc.vector.tensor_reduce(st[:, 0:1], src[:], axis=AX.X,
                                            op=ALU.add)
                    part = bn.tile([n, 4], F32, name=f"part{n}_{len(_dbg)}")
                    _dbg.append(0)
                    for q in range(4):
                        nc.vector.scalar_tensor_tensor(
                            scr[0:n, :], src[:, q * 512:(q + 1) * 512], 1.0,
                            src[:, q * 512:(q + 1) * 512],
                            op0=ALU.mult, op1=ALU.mult,
                            accum_out=part[:, q:q + 1])
                    nc.vector.tensor_reduce(st[:, 1:2], part[:], axis=AX.X,
                                            op=ALU.add)

                _dbg = []
                st1 = bn.tile([H1, 2], F32)
                colstats(a1, H1, st1)
                nc.vector.tensor_reduce(ns2[:, 0:1], nsums[:], axis=AX.X,
                                        op=ALU.add)
                nc.vector.tensor_reduce(ns2[:, 1:2], nsq[:], axis=AX.X,
                                        op=ALU.add)
                pns = bnps.tile([2, 1], F32, tag="s")
                nc.tensor.matmul(pns[:], ns2[:], ones[:, 0:1],
                                 start=True, stop=True)
                nc.vector.tensor_copy(nst2[:], pns[:])

                ar1_in = dp.tile([2 * H1 + 2], F32)
                ar1_out = dp.tile([2 * H1 + 2], F32, addr_space="Shared")
                nc.sync.dma_start(
                    out=ar1_in[0:2 * H1].rearrange("(p f) -> p f", f=2),
                    in_=st1[:])
                nc.sync.dma_start(
                    out=ar1_in[2 * H1:2 * H1 + 2].rearrange("(p f) -> p f", f=1),
                    in_=nst2[:])
                nc.gpsimd.collective_compute(
                    "AllReduce", ALU.add, replica_groups=RG,
                    ins=[ar1_in.opt()], outs=[ar1_out.opt()])
                st1g = bn.tile([H1, 2], F32)
                nstg = bn.tile([1, 2], F32)
                nc.sync.dma_start(
                    out=st1g[:],
                    in_=ar1_out[0:2 * H1].rearrange("(p f) -> p f", f=2))
                nc.sync.dma_start(
                    out=nstg[:],
                    in_=ar1_out[2 * H1:2 * H1 + 2].rearrange("(p f) -> p f", p=1))

                # sampled moments: true sums ~ MOM_STRIDE * sampled sums
                t1 = bn.tile([1, 1], F32)
                t2 = bn.tile([1, 1], F32)
                nc.vector.tensor_tensor(t1[:], nstg[:, 0:1], nstg[:, 0:1],
                                        op=ALU.mult)
                nc.vector.tensor_scalar(t1[:], t1[:],
                                        float(MOM_STRIDE * MOM_STRIDE) / N_ELEMS,
                                        None, op0=ALU.mult)
                nc.vector.tensor_scalar(t2[:], nstg[:, 1:2], float(MOM_STRIDE),
                                        t1[:], op0=ALU.mult, op1=ALU.subtract)
                nc.vector.tensor_scalar(t1[:], t2[:], 1e-5 / (N_ELEMS - 1.0),
                                        None, op0=ALU.mult)
                peps = bnps.tile([H1, 1], F32, tag="s")
                nc.tensor.matmul(peps[:], ones[0:1, 0:H1], t1[:],
                                 start=True, stop=True)

                def bn_affine(stg, gt, btt, n, eps_ap=None, eps_imm=None):
                    k = len(_dbg)
                    _dbg.append(0)
                    mu = bn.tile([n, 1], F32, name=f"mu_{k}")
                    var = bn.tile([n, 1], F32, name=f"var_{k}")
                    sc = bn.tile([n, 1], F32, name=f"sc_{k}")
                    bi = bn.tile([n, 1], F32, name=f"bi_{k}")
                    t = bn.tile([n, 1], F32, name=f"tt_{k}")
                    nc.vector.tensor_scalar(mu[:], stg[:, 0:1], 1.0 / B, None,
                                            op0=ALU.mult)
                    nc.vector.tensor_tensor(t[:], mu[:], mu[:], op=ALU.mult)
                    nc.vector.tensor_scalar(var[:], stg[:, 1:2], 1.0 / B, t[:],
                                            op0=ALU.mult, op1=ALU.subtract)
                    if eps_ap is not None:
                        nc.vector.tensor_tensor(var[:], var[:], eps_ap,
                                                op=ALU.add)
                    else:
                        nc.vector.tensor_scalar(var[:], var[:], eps_imm, None,
                                                op0=ALU.add)
                    nc.scalar.sqrt(t[:], var[:])
                    nc.vector.reciprocal(t[:], t[:])
                    nc.vector.tensor_tensor(sc[:], t[:], gt[:], op=ALU.mult)
                    nc.vector.tensor_tensor(t[:], mu[:], sc[:], op=ALU.mult)
                    nc.vector.tensor_tensor(bi[:], btt[:], t[:],
                                            op=ALU.subtract)
                    return sc, bi

                sc1, bi1 = bn_affine(st1g, g1t, bt1t, H1, eps_ap=peps[:])
                h1 = bn.tile([H1, R], F32R)
                nc.scalar.activation(h1[:], a1[:], ACTF.Relu, bias=bi1[:],
                                     scale=sc1[:])

                # ============ layer 2 ============
                a2 = bn.tile([H2, R], F32)
                for blk in range(4):
                    pa2 = bnps.tile([H2, 512], F32, tag="l")
                    nc.tensor.matmul(pa2[:], wenc[:],
                                     h1[:, blk * 512:(blk + 1) * 512],
                                     start=True, stop=True)
                    nc.vector.tensor_copy(a2[:, blk * 512:(blk + 1) * 512],
                                          pa2[:])
                st2 = bn.tile([H2, 2], F32)
                colstats(a2, H2, st2)
                ar2_in = dp.tile([2 * H2], F32)
                ar2_out = dp.tile([2 * H2], F32, addr_space="Shared")
                nc.sync.dma_start(
                    out=ar2_in[:].rearrange("(p f) -> p f", f=2), in_=st2[:])
                nc.gpsimd.collective_compute(
                    "AllReduce", ALU.add, replica_groups=RG,
                    ins=[ar2_in.opt()], outs=[ar2_out.opt()])
                st2g = bn.tile([H2, 2], F32)
                nc.sync.dma_start(
                    out=st2g[:], in_=ar2_out[:].rearrange("(p f) -> p f", f=2))
                sc2, bi2 = bn_affine(st2g, g2t, bt2t, H2, eps_imm=1e-5)
                h2 = bn.tile([H2, R], F32R)
                nc.scalar.activation(h2[:], a2[:], ACTF.Relu, bias=bi2[:],
                                     scale=sc2[:])

                # ============ layer 3 ============
                a3 = bn.tile([H1, R], F32)
                for blk in range(4):
                    pa3 = bnps.tile([H1, 512], F32, tag="l")
                    nc.tensor.matmul(pa3[:], wdec[:],
                                     h2[:, blk * 512:(blk + 1) * 512],
                                     start=True, stop=True)
                    nc.vector.tensor_copy(a3[:, blk * 512:(blk + 1) * 512],
                                          pa3[:])
                st3 = bn.tile([H1, 2], F32)
                colstats(a3, H1, st3)
                ar3_in = dp.tile([2 * H1], F32)
                ar3_out = dp.tile([2 * H1], F32, addr_space="Shared")
                nc.sync.dma_start(
                    out=ar3_in[:].rearrange("(p f) -> p f", f=2), in_=st3[:])
                nc.gpsimd.collective_compute(
                    "AllReduce", ALU.add, replica_groups=RG,
                    ins=[ar3_in.opt()], outs=[ar3_out.opt()])
                st3g = bn.tile([H1, 2], F32)
                nc.sync.dma_start(
                    out=st3g[:], in_=ar3_out[:].rearrange("(p f) -> p f", f=2))
                sc3, bi3 = bn_affine(st3g, g3t, bt3t, H1, eps_imm=1e-5)
                h3e = sp.tile([H1 + 1, R], F32R)
                nc.vector.memset(h3e[H1:H1 + 1, :].bitcast(F32), 1.0)
                nc.scalar.activation(h3e[0:H1, :], a3[:], ACTF.Relu,
                                     bias=bi3[:], scale=sc3[:])

            # ============ heads (head-major: one ACT LUT per head) ============
            funcs = [ACTF.Sigmoid, ACTF.Exp, ACTF.Exp]
            with tc.tile_pool(name="hpool", bufs=2) as hp, \
                 tc.tile_pool(name="hps", bufs=3, space="PSUM") as hps:
                for h in range(3):
                    for t in range(NT):
                        for cc2 in range(2):
                            ot = hp.tile([128, 2048], F32, tag="o")
                            for half in range(2):
                                ph = hps.tile([128, 1024], F32, tag="h")
                                for q in range(2):
                                    cc = 4 * cc2 + 2 * half + q
                                    nc.tensor.matmul(
                                        ph[:, q * 512:(q + 1) * 512],
                                        h3e[:, t * 128:(t + 1) * 128],
                                        whe[:, h, cc * 512:(cc + 1) * 512],
                                        start=True, stop=True)
                                nc.scalar.activation(
                                    ot[:, half * 1024:(half + 1) * 1024],
                                    ph[:], funcs[h])
                            nc.sync.dma_start(
                                out=out_d[h][t * 128:(t + 1) * 128,
                                             cc2 * 2048:(cc2 + 1) * 2048],
                                in_=ot[:])

    nc.compile()
    return nc


def _consts():
    return {
        "ident": np.eye(128, dtype=np.float32),
        "ones": np.ones((128, 128), dtype=np.float32),
        "j15": np.tile(np.arange(1, 16, dtype=np.float32), (128, 1)),
    }


LAST_RESULT = None


def kernel(**inputs):
    global LAST_RESULT
    if "nc" not in _CACHE:
        _CACHE["nc"] = _build()
    nc = _CACHE["nc"]

    np_in = {k: np.asarray(v, dtype=np.float32) for k, v in inputs.items()}
    x = np_in["x"]
    shared = {k: np_in[k] for k in np_in if k != "x"}
    shared.update(_consts())
    in_maps = []
    for c in range(N_CORES):
        m = dict(shared)
        m["x"] = np.ascontiguousarray(x[c * R:(c + 1) * R])
        in_maps.append(m)

    res = run_bass_kernel_spmd(nc, in_maps, core_ids=list(range(N_CORES)))
    LAST_RESULT = res
    pi = np.concatenate([res.results[c]["PI"] for c in range(N_CORES)], axis=0)
    m_ = np.concatenate([res.results[c]["M"] for c in range(N_CORES)], axis=0)
    th = np.concatenate([res.results[c]["TH"] for c in range(N_CORES)], axis=0)
    return (pi, m_, th)
